# revision 3
# baseline (speedup 1.0000x reference)
"""Multi-head attention (16 heads, d=64, d_model=1024, SL=2048, BS=2) on 8
Trainium2 NeuronCores — v2.

Sharding: core c handles batch b = c // 4 and heads [4*(c%4), 4*(c%4)+4).
Each core computes a partial output y_c[2048, 1024]; host sums 4 partials
per batch.

v2 changes vs baseline:
- QT/KT stored bf16 (scores matmuls stream bf16; FWL on stationary).
- Scores matmuls avoid the slow (64,128)-tiled mode: "padded" variant uses
  zero-padded [128,128] stationaries at full rate; "tiled" uses the 2-tile
  row mode (only if HW shows it's fast for bf16).
- Startup software pipeline: piecewise input DMAs (token halves) with
  priority K+Q first-half -> V -> K second -> V second -> Q second;
  projections interleaved into the attention instruction stream so the
  ACT engine (softmax exp, the throughput floor) starts ~13us in.
- exp consumes [128,1024] PSUM tiles covering a head-pair per k-tile; AV
  runs one k-tile behind; P stored bf16.
- o-proj for chunk qc deferred into chunk qc+1's score stream.
"""

import os
import sys
for _p in ("/opt/trn_rl_repo", "/root/.axon_site/_ro/trn_rl_repo"):
    if os.path.isdir(_p) and _p not in sys.path:
        sys.path.insert(0, _p)

import numpy as np

import concourse.bass as bass
import concourse.tile as tile
from concourse import bacc, mybir
from concourse.bass_utils import run_bass_kernel_spmd

N_CORES = 8
SL = 2048
BS = 2
DM = 1024
H = 16
DH = 64
HPC = 4
IC = HPC * DH          # 256
F32 = mybir.dt.float32
BF16 = mybir.dt.bfloat16
F32R = mybir.dt.float32r
Exp = mybir.ActivationFunctionType.Exp

N_DMC = DM // 128      # 8
N_KT = SL // 128       # 16
N_QC = SL // 512       # 4
VW = 65
VBLK = HPC * VW        # 260

SCORES_VARIANT = "padded"
# padded: no tiled-mode matmuls anywhere -> f32r activations (baseline
# numerics). tiled: (64,128) row-tiled scores in groups of 4 -> bf16
# streams (the configuration measured fast on HW).
SDT = mybir.dt.float32r if SCORES_VARIANT == "padded" else mybir.dt.bfloat16


def build_kernel():
    nc = bacc.Bacc("TRN2", target_bir_lowering=False, debug=False,
                   num_devices=N_CORES)
    qT = nc.dram_tensor("qT", [DM, SL], BF16, kind="ExternalInput").ap()
    kT = nc.dram_tensor("kT", [DM, SL], BF16, kind="ExternalInput").ap()
    vT = nc.dram_tensor("vT", [DM, SL], BF16, kind="ExternalInput").ap()
    wqT = nc.dram_tensor("wqT", [DM, IC], BF16, kind="ExternalInput").ap()
    wkT = nc.dram_tensor("wkT", [DM, IC], BF16, kind="ExternalInput").ap()
    wvT = nc.dram_tensor("wvT", [DM, IC], BF16, kind="ExternalInput").ap()
    woT = nc.dram_tensor("woT", [IC, DM], F32R, kind="ExternalInput").ap()
    Y = nc.dram_tensor("Y", [SL, DM], F32, kind="ExternalOutput").ap()
    with tile.TileContext(nc) as tc:
        _build_body(nc, tc, qT, kT, vT, wqT, wkT, wvT, woT, Y)
    nc.compile()
    return nc


def _build_body(nc, tc, qT, kT, vT, wqT, wkT, wvT, woT, Y):
    import contextlib
    ctx = contextlib.ExitStack()
    with ctx:
        wpool = ctx.enter_context(tc.tile_pool(name="w", bufs=1))
        xin = ctx.enter_context(tc.tile_pool(name="xin", bufs=28))
        qk = ctx.enter_context(tc.tile_pool(name="qk", bufs=1))
        vpool = ctx.enter_context(tc.tile_pool(name="v", bufs=1))
        ptp = ctx.enter_context(tc.tile_pool(name="pt", bufs=6))
        atp = ctx.enter_context(tc.tile_pool(name="at", bufs=1))
        ypool = ctx.enter_context(tc.tile_pool(name="y", bufs=2))
        misc = ctx.enter_context(tc.tile_pool(name="misc", bufs=2))
        ps = ctx.enter_context(tc.tile_pool(name="ps", bufs=2, space="PSUM"))
        pau = ctx.enter_context(tc.tile_pool(name="pau", bufs=2, space="PSUM"))
        pacc = ctx.enter_context(tc.tile_pool(name="pacc", bufs=2, space="PSUM"))

        # ---- weight tiles + DMA (scalar queue; ACT idle at start) ----
        w_sb = {}
        w_dram = {"wq": wqT, "wk": wkT, "wv": wvT}
        for name in ("wq", "wk", "wv"):
            t = wpool.tile([128, N_DMC * IC], BF16, tag=name, name=name)
            w_sb[name] = t
            nc.scalar.dma_start(
                out=t[:].rearrange("p (c f) -> p c f", c=N_DMC),
                in_=w_dram[name].rearrange("(c p) f -> p c f", p=128))
        wo_sb = []
        for i in range(2):
            t = wpool.tile([128, DM], F32R, tag=f"wo{i}", name=f"wo{i}")
            wo_sb.append(t)
            nc.scalar.dma_start(out=t[:], in_=woT[i * 128:(i + 1) * 128, :])

        ones_f32 = misc.tile([128, DH], F32, tag="ones_f32")
        nc.vector.memset(ones_f32[:], 1.0)

        # ---- input DMA pieces: [128, 1024] halves, priority-ordered ----
        # piece key: (tensor, c, half)
        # Both issuing queues below are HWDGE (sync=SP, scalar=ACT);
        # gpsimd's SWDGE path measured far slower triggers, keep it for
        # Y stores only.  sync carries K then Q (gates the first scores);
        # scalar carries V (needed ~AV time) after the weights.
        xdram = {"q": qT, "k": kT, "v": vT}
        xtiles = {}  # (tensor, c, half) -> tile

        def load_piece(eng, tn, c, h):
            t = xin.tile([128, SL // 2], BF16, tag="xin",
                         name=f"x_{tn}{c}h{h}")
            xtiles[(tn, c, h)] = t
            eng.dma_start(
                out=t[:],
                in_=xdram[tn][c * 128:(c + 1) * 128,
                              h * (SL // 2):(h + 1) * (SL // 2)])

        for c in range(N_DMC):
            load_piece(nc.sync, "k", c, 0)
        for c in range(N_DMC):
            load_piece(nc.scalar, "v", c, 0)
        for c in range(N_DMC):
            load_piece(nc.sync, "q", c, 0)
        for c in range(N_DMC):
            load_piece(nc.sync, "k", c, 1)
        for c in range(N_DMC):
            load_piece(nc.scalar, "v", c, 1)
        for c in range(N_DMC):
            load_piece(nc.sync, "q", c, 1)

        # ---- long-lived activation tiles ----
        QT = [qk.tile([128, SL], SDT, tag=f"qt{p}", name=f"qt{p}")
              for p in range(2)]
        if SCORES_VARIANT == "padded":
            # KTz[pair][parity]: [128, SL] with only rows
            # parity*64:(parity+1)*64 live; complementary rows zero.
            KTz = [[qk.tile([128, SL], SDT, tag=f"ktz{p}{e}",
                            name=f"ktz{p}{e}") for e in range(2)]
                   for p in range(2)]
            for p in range(2):
                nc.vector.memset(KTz[p][0][64:128, :].bitcast(F32) if SDT == F32R else KTz[p][0][64:128, :], 0.0)
                nc.vector.memset(KTz[p][1][0:64, :].bitcast(F32) if SDT == F32R else KTz[p][1][0:64, :], 0.0)
        else:
            KT = [qk.tile([128, SL], SDT, tag=f"kt{p}", name=f"kt{p}")
                  for p in range(2)]
        AT = [atp.tile([128, SL], F32R, tag=f"at{p}", name=f"at{p}")
              for p in range(2)]
        V = vpool.tile([128, N_KT * VBLK], SDT, tag="vsb")
        for h in range(HPC):
            nc.vector.tensor_copy(V[:, h * VW + 64::VBLK],
                                  ones_f32[:, 0:N_KT])

        # ---- projection helpers (per 512-token chunk) ----
        def proj_qk_tcq(tn, wname, tcq):
            """Project tokens [tcq*512,(tcq+1)*512) of q or k."""
            h = tcq // 2
            off = (tcq % 2) * 512
            for hp in range(2):
                acc = pacc.tile([128, 512], F32, tag="acc")
                for c in range(N_DMC):
                    nc.tensor.matmul(
                        acc[:],
                        w_sb[wname][:, c * IC + hp * 128:
                                    c * IC + (hp + 1) * 128],
                        xtiles[(tn, c, h)][:, off:off + 512],
                        start=(c == 0), stop=(c == N_DMC - 1))
                if tn == "q":
                    nc.vector.tensor_copy(
                        QT[hp][:, tcq * 512:(tcq + 1) * 512], acc[:])
                elif SCORES_VARIANT == "padded":
                    for e in range(2):
                        nc.vector.tensor_copy(
                            KTz[hp][e][e * 64:(e + 1) * 64,
                                       tcq * 512:(tcq + 1) * 512],
                            acc[e * 64:(e + 1) * 64, :])
                else:
                    nc.vector.tensor_copy(
                        KT[hp][:, tcq * 512:(tcq + 1) * 512], acc[:])

        def proj_v_kt(kt):
            """Project V for k-tile kt (128 tokens)."""
            h = kt // 8
            off = (kt % 8) * 128
            acc = pacc.tile([128, 512], F32, tag="acc")
            for c in range(N_DMC):
                nc.tensor.matmul(
                    acc[:, 0:IC],
                    xtiles[("v", c, h)][:, off:off + 128],
                    w_sb["wv"][:, c * IC:(c + 1) * IC],
                    start=(c == 0), stop=(c == N_DMC - 1))
            for hh in range(HPC):
                nc.vector.tensor_copy(
                    V[:, kt * VBLK + hh * VW:kt * VBLK + hh * VW + 64],
                    acc[:, hh * 64:(hh + 1) * 64])

        # ---- attention pieces ----
        def scores_mm(qc, pair, kt, s):
            if SCORES_VARIANT == "padded":
                for e in range(2):
                    nc.tensor.matmul(
                        s[:, e * 512:(e + 1) * 512],
                        KTz[pair][e][:, kt * 128:(kt + 1) * 128],
                        QT[pair][:, qc * 512:(qc + 1) * 512],
                        start=True, stop=True)
            else:
                for e in range(2):
                    nc.tensor.matmul(
                        s[:, e * 512:(e + 1) * 512],
                        KT[pair][e * 64:(e + 1) * 64,
                                 kt * 128:(kt + 1) * 128],
                        QT[pair][e * 64:(e + 1) * 64,
                                 qc * 512:(qc + 1) * 512],
                        start=True, stop=True)

        def av_mm(pair, kt, p, au):
            for hl in range(2):
                hh = pair * 2 + hl
                nc.tensor.matmul(
                    au[hl][:],
                    V[:, kt * VBLK + hh * VW:kt * VBLK + (hh + 1) * VW],
                    p[:, hl * 512:(hl + 1) * 512],
                    start=(kt == 0), stop=(kt == N_KT - 1))

        def normalize(qc, pair, au):
            for hl in range(2):
                l_sb = misc.tile([1, 512], F32, tag="l_sb")
                nc.vector.tensor_copy(l_sb[:], au[hl][64:65, :])
                rc = misc.tile([1, 512], F32, tag="rc")
                nc.vector.reciprocal_approx_fast(out=rc[:], in_=l_sb[:])
                rb = misc.tile([64, 512], F32, tag="rb")
                nc.gpsimd.partition_broadcast(rb[:], rc[:])
                nc.vector.tensor_mul(
                    AT[pair][hl * 64:(hl + 1) * 64,
                             qc * 512:(qc + 1) * 512],
                    au[hl][0:64, :], rb[:])

        def oproj_qt(qt):
            if True:
                y_sb = ypool.tile([128, DM], F32, tag="ysb")
                for mh in range(2):
                    yp = pacc.tile([128, 512], F32, tag="acc")
                    for ich in range(2):
                        nc.tensor.matmul(
                            yp[:],
                            AT[ich][:, qt * 128:(qt + 1) * 128],
                            wo_sb[ich][:, mh * 512:(mh + 1) * 512],
                            start=(ich == 0), stop=(ich == 1))
                    nc.vector.tensor_copy(y_sb[:, mh * 512:(mh + 1) * 512],
                                          yp[:])
                nc.gpsimd.dma_start(out=Y[qt * 128:(qt + 1) * 128, :],
                                    in_=y_sb[:])

        # ---- issue schedule ----
        # Startup: K/Q proj of first token-half, then qc0 attention with
        # V-proj and remaining projections woven into the score stream.
        proj_qk_tcq("k", "wk", 0)
        proj_qk_tcq("q", "wq", 0)

        pending_oproj = None
        for qc in range(N_QC):
            for pair in range(2):
                # shorter AV pipeline on the final pair to shrink the tail
                AV_LAG = 1 if (qc == N_QC - 1 and pair == 1) else 4
                au = [pau.tile([VW, 512], F32, tag="au", name=f"au{hl}")
                      for hl in range(2)]
                pend = []  # (kt, p-tile) awaiting AV, issued AV_LAG behind
                for kt in range(N_KT):
                    s = ps.tile([128, 1024], F32, tag="s")
                    scores_mm(qc, pair, kt, s)
                    # weave deferred work into the score stream.  In qc0/p0
                    # the V projections must precede the (lagged) AVs that
                    # consume them: Vproj(0-7)@kt3 < av(kt0)@kt4;
                    # Vproj(8-11)@kt12 < av(kt8)@kt12's pop; Vproj(12-15)
                    # @kt14 < av(kt12)@tail.
                    if qc == 0 and pair == 0:
                        if kt == 0:
                            proj_qk_tcq("k", "wk", 1)
                        elif kt == 1:
                            proj_qk_tcq("q", "wq", 1)
                        elif kt == 3:
                            for k2 in range(0, 8):
                                proj_v_kt(k2)
                        elif kt == 7:
                            proj_qk_tcq("k", "wk", 2)
                        elif kt == 9:
                            proj_qk_tcq("k", "wk", 3)
                        elif kt == 12:
                            for k2 in range(8, 12):
                                proj_v_kt(k2)
                        elif kt == 14:
                            for k2 in range(12, N_KT):
                                proj_v_kt(k2)
                    elif qc == 0 and pair == 1 and kt == 5:
                        proj_qk_tcq("q", "wq", 2)
                    elif qc == 0 and pair == 1 and kt == 7:
                        proj_qk_tcq("q", "wq", 3)
                    elif (pair == 0 and kt in (2, 4, 6, 8)
                          and pending_oproj is not None):
                        oproj_qt(4 * pending_oproj + (kt - 2) // 2)
                        if kt == 8:
                            pending_oproj = None
                    p = ptp.tile([128, 1024], SDT, tag="pt")
                    nc.scalar.activation(p[:], s[:], Exp)
                    pend.append((kt, p))
                    if SCORES_VARIANT == "tiled":
                        # batch AVs in pairs so the PE stream alternates
                        # groups of 4 tiled and 4 full matmuls (mode-switch
                        # cost measured acceptable at >=4-groups).
                        if kt % 2 == 1:
                            while len(pend) > AV_LAG:
                                k2, p2 = pend.pop(0)
                                av_mm(pair, k2, p2, au)
                    elif len(pend) > AV_LAG:
                        k2, p2 = pend.pop(0)
                        av_mm(pair, k2, p2, au)
                for k2, p2 in pend:
                    av_mm(pair, k2, p2, au)
                normalize(qc, pair, au)
            pending_oproj = qc
        for qt in range(4 * pending_oproj, 4 * (pending_oproj + 1)):
            oproj_qt(qt)


_NC_CACHE = None


def _get_nc():
    global _NC_CACHE
    if _NC_CACHE is None:
        _NC_CACHE = build_kernel()
    return _NC_CACHE


def make_in_maps(query, keys, values, Wq, Wk, Wv, Wo):
    query = np.ascontiguousarray(query, dtype=np.float32)
    keys = np.ascontiguousarray(keys, dtype=np.float32)
    values = np.ascontiguousarray(values, dtype=np.float32)
    import ml_dtypes
    bf16 = ml_dtypes.bfloat16
    xTs = {}
    for b in range(BS):
        xTs[b] = (
            np.ascontiguousarray(query[:, b, :].T.astype(bf16)),
            np.ascontiguousarray(keys[:, b, :].T.astype(bf16)),
            np.ascontiguousarray(values[:, b, :].T.astype(bf16)),
        )
    wTs = {}
    for g in range(N_CORES // BS):
        sl = slice(g * IC, (g + 1) * IC)
        wTs[g] = (
            np.ascontiguousarray(np.asarray(Wq, np.float32)[sl, :].T.astype(bf16)),
            np.ascontiguousarray(np.asarray(Wk, np.float32)[sl, :].T.astype(bf16)),
            np.ascontiguousarray(np.asarray(Wv, np.float32)[sl, :].T.astype(bf16)),
            np.ascontiguousarray(np.asarray(Wo, np.float32)[:, sl].T),
        )
    in_maps = []
    for c in range(N_CORES):
        b, g = c // 4, c % 4
        qTb, kTb, vTb = xTs[b]
        wq, wk, wv, wo = wTs[g]
        in_maps.append({"qT": qTb, "kT": kTb, "vT": vTb,
                        "wqT": wq, "wkT": wk, "wvT": wv, "woT": wo})
    return in_maps


def assemble_output(results):
    out = np.zeros((SL, BS, DM), dtype=np.float32)
    for c in range(N_CORES):
        b = c // 4
        out[:, b, :] += results[c]["Y"]
    return out


def kernel(query, keys, values, Wq, Wk, Wv, Wo):
    nc = _get_nc()
    in_maps = make_in_maps(query, keys, values, Wq, Wk, Wv, Wo)
    res = run_bass_kernel_spmd(nc, in_maps, list(range(N_CORES)))
    return assemble_output(res.results)
